# revision 8
# baseline (speedup 1.0000x reference)
"""Trainium2 Bass kernel for a fused GRU cell.

Reference computation (B=4096, IN=1024, H=1024, all fp32):
    x_proj = x @ W_ih.T + b_ih            # (B, 3H)
    r_x, z_x, n_x = split(x_proj, 3)
    rz_h = h @ W_rzh.T                    # (B, 2H)
    r = sigmoid(r_x + r_h); z = sigmoid(z_x + z_h)
    n = tanh(n_x + r * (h @ W_nh.T + b_nh))
    out = (1-z)*n + z*h

Strategy:
  - Data-parallel over batch across 8 NeuronCores (512 rows each);
    weights replicated (packed host-side into PE-friendly tiles).
  - Transposed layout on chip: features on partitions, batch on the free
    dim, so per-feature biases are per-partition ACT activation biases.
  - r/z projections fused into ONE K=2048 contraction by concatenating
    [x;h] and [W_ih[:2H].T; W_rzh.T] host-side.
  - Matmuls in fp16 (1 cycle/row on PE, 2 bytes of HBM traffic) with
    fp32 PSUM accumulation; everything else fp32.
"""

import numpy as np

import concourse.bass as bass
import concourse.mybir as mybir
import concourse.tile as tile
from concourse import bacc
from concourse.bass_utils import run_bass_kernel_spmd

B, IN, H = 4096, 1024, 1024
NCORES = 8
BC = B // NCORES          # 512 batch rows per core
P = 128

KO_RZ = (IN + H) // P     # 16 contraction subtiles for the fused r/z matmul
G_RZ = 2 * H // P         # 16 gate tiles (0..7 = r, 8..15 = z)
KO_N = IN // P            # 8
G_N = H // P              # 8

F16 = mybir.dt.float16
F32 = mybir.dt.float32
AF = mybir.ActivationFunctionType
ALU = mybir.AluOpType


def build_bass():
    """Build the per-core Bass program (identical on all cores)."""
    nc = bacc.Bacc("TRN2", target_bir_lowering=False, debug=False)

    xh_d = nc.dram_tensor("xh", [P, KO_RZ, BC], F16, kind="ExternalInput")
    hf_d = nc.dram_tensor("hf", [P, G_N, BC], F32, kind="ExternalInput")
    wrz_d = nc.dram_tensor("wrz", [G_RZ, P, KO_RZ, P], F16, kind="ExternalInput")
    wnx_d = nc.dram_tensor("wnx", [G_N, P, KO_N, P], F16, kind="ExternalInput")
    wnh_d = nc.dram_tensor("wnh", [G_N, P, KO_N, P], F16, kind="ExternalInput")
    brz_d = nc.dram_tensor("brz", [P, G_RZ], F32, kind="ExternalInput")
    bn_d = nc.dram_tensor("bn", [P, G_N], F32, kind="ExternalInput")
    bnh_d = nc.dram_tensor("bnh", [P, G_N], F32, kind="ExternalInput")
    out_d = nc.dram_tensor("outp", [P, G_N, BC], F32, kind="ExternalOutput")

    with tile.TileContext(nc) as tc:
        with (
            tc.tile_pool(name="const", bufs=1) as cpool,
            tc.tile_pool(name="wrzp", bufs=4) as wrzp,
            tc.tile_pool(name="wnp", bufs=3) as wnp,
            tc.tile_pool(name="rzp", bufs=1) as rzp,
            tc.tile_pool(name="tmp", bufs=4) as tp,
            tc.tile_pool(name="ps_rz", bufs=3, space="PSUM") as pp_rz,
            tc.tile_pool(name="ps_x", bufs=2, space="PSUM") as pp_x,
            tc.tile_pool(name="ps_h", bufs=2, space="PSUM") as pp_h,
        ):
            # DMA issue order matters: transfers complete roughly in issue
            # order across the HWDGE queues, and the first matmul waits on
            # the first weight tile — so issue it before the activations.
            # Startup critical path: the first matmul needs only the first
            # ko-chunk of the g=0 weight tile and the first xh chunk, so
            # split those DMAs (deps are view-overlap-based, so matmuls on
            # a ko slice wait only for the chunk that covers it).
            w0 = wrzp.tile([P, KO_RZ, P], F16, tag="wrz")
            nc.sync.dma_start(out=w0[:, 0:4, :], in_=wrz_d[0, :, 0:4, :])
            XH_CH = 4
            xh_chunks = []
            for c in range(KO_RZ // XH_CH):
                t = cpool.tile([P, XH_CH, BC], F16, tag=f"xh{c}")
                if c == 0:
                    nc.sync.dma_start(out=t[:, 0:2, :], in_=xh_d[:, 0:2, :])
                    nc.sync.dma_start(out=w0[:, 4:, :], in_=wrz_d[0, :, 4:, :])
                    nc.sync.dma_start(out=t[:, 2:4, :], in_=xh_d[:, 2:4, :])
                else:
                    nc.sync.dma_start(
                        out=t[:], in_=xh_d[:, c * XH_CH:(c + 1) * XH_CH, :]
                    )
                xh_chunks.append(t)
            xh_sb = [
                xh_chunks[ko // XH_CH][:, ko % XH_CH, :] for ko in range(KO_RZ)
            ]
            brz_sb = cpool.tile([P, G_RZ], F32, tag="brz")
            nc.sync.dma_start(out=brz_sb[:], in_=brz_d[:])

            # Phase B: fused r/z projection, 16 gate tiles x K=2048.
            # Tiles 0..7 produce r (kept as-is); tiles 8..15 produce z,
            # from which we precompute (1-z) and z*h so the phase-C tail
            # after the last matmul is as short as possible.
            rz_blk = rzp.tile([P, G_RZ, BC], F32, tag="rzblk")
            omz_blk = rzp.tile([P, G_N, BC], F32, tag="omzblk")
            zh_blk = rzp.tile([P, G_N, BC], F32, tag="zhblk")
            hf_sb = rzp.tile([P, G_N, BC], F32, tag="hfblk")
            for g in range(G_RZ):
                if g == 0:
                    w = w0
                else:
                    w = wrzp.tile([P, KO_RZ, P], F16, tag="wrz")
                    nc.sync.dma_start(out=w[:], in_=wrz_d[g])
                if g == 4 or g == 6:
                    # fp32 h halves, needed from the z tiles (g >= 8) onward
                    half = (g - 4) // 2
                    nc.sync.dma_start(
                        out=hf_sb[:, half * 4:(half + 1) * 4, :],
                        in_=hf_d[:, half * 4:(half + 1) * 4, :],
                    )
                ps = pp_rz.tile([P, BC], F32, tag="psrz")
                for ko in range(KO_RZ):
                    nc.tensor.matmul(
                        ps[:], w[:, ko, :], xh_sb[ko],
                        start=(ko == 0), stop=(ko == KO_RZ - 1),
                    )
                rz = rz_blk[:, g, :]
                nc.scalar.activation(
                    rz, ps[:], AF.Sigmoid, bias=brz_sb[:, g:g + 1]
                )
                if g >= G_N:
                    gz = g - G_N
                    nc.vector.tensor_scalar(
                        omz_blk[:, gz, :], rz, -1.0, 1.0,
                        op0=ALU.mult, op1=ALU.add,
                    )
                    nc.vector.tensor_mul(
                        out=zh_blk[:, gz, :], in0=rz, in1=hf_sb[:, gz, :]
                    )

            bn_sb = cpool.tile([P, G_N], F32, tag="bn")
            nc.sync.dma_start(out=bn_sb[:], in_=bn_d[:])
            bnh_sb = cpool.tile([P, G_N], F32, tag="bnh")
            nc.sync.dma_start(out=bnh_sb[:], in_=bnh_d[:])

            # Phase C: n gate + output blend, 8 gate tiles.
            # psh is accumulated first so the (psh + b_nh) * r fusion can
            # overlap the psx matmuls; only the += psx sits after the
            # final matmul of each tile.
            for g in range(G_N):
                wh = wnp.tile([P, KO_N, P], F16, tag="wnh")
                nc.sync.dma_start(out=wh[:], in_=wnh_d[g])
                wx = wnp.tile([P, KO_N, P], F16, tag="wnx")
                nc.sync.dma_start(out=wx[:], in_=wnx_d[g])
                psx = pp_x.tile([P, BC], F32, tag="psx")
                psh = pp_h.tile([P, BC], F32, tag="psh")
                for ko in range(KO_N):
                    nc.tensor.matmul(
                        psh[:], wh[:, ko, :], xh_sb[KO_N + ko],
                        start=(ko == 0), stop=(ko == KO_N - 1),
                    )
                for ko in range(KO_N):
                    nc.tensor.matmul(
                        psx[:], wx[:, ko, :], xh_sb[ko],
                        start=(ko == 0), stop=(ko == KO_N - 1),
                    )
                # t = (psh + b_nh) * r      (overlaps the psx matmuls)
                t = tp.tile([P, BC], F32, tag="t")
                nc.vector.scalar_tensor_tensor(
                    t[:], psh[:], bnh_sb[:, g:g + 1], rz_blk[:, g, :],
                    op0=ALU.add, op1=ALU.mult,
                )
                # t += psx
                nc.vector.tensor_add(out=t[:], in0=t[:], in1=psx[:])
                # n = tanh(t + b_n)
                n_t = tp.tile([P, BC], F32, tag="n")
                nc.scalar.activation(
                    n_t[:], t[:], AF.Tanh, bias=bn_sb[:, g:g + 1]
                )
                # out = n*(1-z) + z*h
                u = tp.tile([P, BC], F32, tag="u")
                nc.vector.tensor_mul(out=u[:], in0=n_t[:], in1=omz_blk[:, g, :])
                o = tp.tile([P, BC], F32, tag="o")
                nc.vector.tensor_add(out=o[:], in0=u[:], in1=zh_blk[:, g, :])
                nc.sync.dma_start(out=out_d[:, g, :], in_=o[:])

    nc.compile()
    return nc


def prepare_inputs(x, h, W_ih, b_ih, W_rzh, W_nh, b_nh):
    """Host-side packing: shard batch, transpose/concat/cast weights."""
    f16 = np.float16
    # Fused r/z weight: (IN+H, 2H) -> [g, p, ko, mi] tile-major
    wrz_cat = np.concatenate(
        [W_ih[: 2 * H].T, W_rzh.T], axis=0
    ).astype(f16)
    wrz = np.ascontiguousarray(
        wrz_cat.reshape(KO_RZ, P, G_RZ, P).transpose(2, 1, 0, 3)
    )
    wnx = np.ascontiguousarray(
        W_ih[2 * H:].T.astype(f16).reshape(KO_N, P, G_N, P).transpose(2, 1, 0, 3)
    )
    wnh = np.ascontiguousarray(
        W_nh.T.astype(f16).reshape(KO_N, P, G_N, P).transpose(2, 1, 0, 3)
    )
    brz = np.ascontiguousarray(b_ih[: 2 * H].reshape(G_RZ, P).T).astype(np.float32)
    bn = np.ascontiguousarray(b_ih[2 * H:].reshape(G_N, P).T).astype(np.float32)
    bnh = np.ascontiguousarray(b_nh.reshape(G_N, P).T).astype(np.float32)

    xh_catT = np.concatenate([x.T, h.T], axis=0).astype(f16)  # (2048, B)
    hT = np.ascontiguousarray(h.T.astype(np.float32))          # (1024, B)

    in_maps = []
    for c in range(NCORES):
        cols = slice(c * BC, (c + 1) * BC)
        xh_c = np.ascontiguousarray(
            xh_catT[:, cols].reshape(KO_RZ, P, BC).transpose(1, 0, 2)
        )
        hf_c = np.ascontiguousarray(
            hT[:, cols].reshape(G_N, P, BC).transpose(1, 0, 2)
        )
        in_maps.append(
            {
                "xh": xh_c,
                "hf": hf_c,
                "wrz": wrz,
                "wnx": wnx,
                "wnh": wnh,
                "brz": brz,
                "bn": bn,
                "bnh": bnh,
            }
        )
    return in_maps


def assemble_output(results):
    """results: list of per-core dicts with 'outp' [P, G_N, BC] fp32."""
    parts = []
    for c in range(NCORES):
        oc = results[c]["outp"]                       # [128, 8, 512]
        ocT = oc.transpose(1, 0, 2).reshape(H, BC)    # features x batch
        parts.append(np.ascontiguousarray(ocT.T))     # batch x features
    return np.concatenate(parts, axis=0).astype(np.float32)


def kernel(x, h, W_ih, b_ih, W_rzh, W_nh, b_nh):
    x = np.asarray(x, dtype=np.float32)
    h = np.asarray(h, dtype=np.float32)
    W_ih = np.asarray(W_ih, dtype=np.float32)
    b_ih = np.asarray(b_ih, dtype=np.float32)
    W_rzh = np.asarray(W_rzh, dtype=np.float32)
    W_nh = np.asarray(W_nh, dtype=np.float32)
    b_nh = np.asarray(b_nh, dtype=np.float32)

    in_maps = prepare_inputs(x, h, W_ih, b_ih, W_rzh, W_nh, b_nh)
    nc = build_bass()
    res = run_bass_kernel_spmd(nc, in_maps, core_ids=list(range(NCORES)))
    return assemble_output(res.results)


# revision 9
# speedup vs baseline: 1.0682x; 1.0682x over previous
"""Trainium2 Bass kernel for a fused GRU cell.

Reference computation (B=4096, IN=1024, H=1024, all fp32):
    x_proj = x @ W_ih.T + b_ih            # (B, 3H)
    r_x, z_x, n_x = split(x_proj, 3)
    rz_h = h @ W_rzh.T                    # (B, 2H)
    r = sigmoid(r_x + r_h); z = sigmoid(z_x + z_h)
    n = tanh(n_x + r * (h @ W_nh.T + b_nh))
    out = (1-z)*n + z*h

Strategy:
  - Data-parallel over batch across 8 NeuronCores (512 rows each);
    weights replicated (packed host-side into PE-friendly tiles).
  - Transposed layout on chip: features on partitions, batch on the free
    dim, so per-feature biases are per-partition ACT activation biases.
  - r/z projections fused into ONE K=2048 contraction by concatenating
    [x;h] and [W_ih[:2H].T; W_rzh.T] host-side.
  - Matmuls in fp16 (1 cycle/row on PE, 2 bytes of HBM traffic) with
    fp32 PSUM accumulation; everything else fp32.
"""

import numpy as np

import concourse.bass as bass
import concourse.mybir as mybir
import concourse.tile as tile
from concourse import bacc
from concourse.bass_utils import run_bass_kernel_spmd

B, IN, H = 4096, 1024, 1024
NCORES = 8
BC = B // NCORES          # 512 batch rows per core
P = 128

KO_RZ = (IN + H) // P     # 16 contraction subtiles for the fused r/z matmul
G_RZ = 2 * H // P         # 16 gate tiles (0..7 = r, 8..15 = z)
KO_N = IN // P            # 8
G_N = H // P              # 8

F16 = mybir.dt.float16
F32 = mybir.dt.float32
AF = mybir.ActivationFunctionType
ALU = mybir.AluOpType


def build_bass():
    """Build the per-core Bass program (identical on all cores)."""
    nc = bacc.Bacc("TRN2", target_bir_lowering=False, debug=False)

    xh_d = nc.dram_tensor("xh", [P, KO_RZ, BC], F16, kind="ExternalInput")
    hf_d = nc.dram_tensor("hf", [P, G_N, BC], F32, kind="ExternalInput")
    wrz_d = nc.dram_tensor("wrz", [G_RZ, P, KO_RZ, P], F16, kind="ExternalInput")
    wnx_d = nc.dram_tensor("wnx", [G_N, P, KO_N, P], F16, kind="ExternalInput")
    wnh_d = nc.dram_tensor("wnh", [G_N, P, KO_N, P], F16, kind="ExternalInput")
    brz_d = nc.dram_tensor("brz", [P, G_RZ], F32, kind="ExternalInput")
    bn_d = nc.dram_tensor("bn", [P, G_N], F32, kind="ExternalInput")
    bnh_d = nc.dram_tensor("bnh", [P, G_N], F32, kind="ExternalInput")
    out_d = nc.dram_tensor("outp", [P, G_N, BC], F32, kind="ExternalOutput")

    with tile.TileContext(nc) as tc:
        with (
            tc.tile_pool(name="const", bufs=1) as cpool,
            tc.tile_pool(name="wrzp", bufs=4) as wrzp,
            tc.tile_pool(name="wnp", bufs=3) as wnp,
            tc.tile_pool(name="rzp", bufs=1) as rzp,
            tc.tile_pool(name="tmp", bufs=4) as tp,
            tc.tile_pool(name="ps_rz", bufs=3, space="PSUM") as pp_rz,
            tc.tile_pool(name="ps_x", bufs=2, space="PSUM") as pp_x,
            tc.tile_pool(name="ps_h", bufs=2, space="PSUM") as pp_h,
        ):
            # DMA issue order matters: transfers complete roughly in issue
            # order across the HWDGE queues, and the first matmul waits on
            # the first weight tile — so issue it before the activations.
            # Startup critical path: the first matmul needs only the first
            # ko-chunk of the g=0 weight tile and the first xh chunk, so
            # split those DMAs (deps are view-overlap-based, so matmuls on
            # a ko slice wait only for the chunk that covers it).
            w0 = wrzp.tile([P, KO_RZ, P], F16, tag="wrz")
            nc.sync.dma_start(out=w0[:, 0:4, :], in_=wrz_d[0, :, 0:4, :])
            XH_CH = 4
            xh_chunks = []
            for c in range(KO_RZ // XH_CH):
                t = cpool.tile([P, XH_CH, BC], F16, tag=f"xh{c}")
                if c == 0:
                    nc.sync.dma_start(out=t[:, 0:2, :], in_=xh_d[:, 0:2, :])
                    nc.sync.dma_start(out=w0[:, 4:, :], in_=wrz_d[0, :, 4:, :])
                    nc.sync.dma_start(out=t[:, 2:4, :], in_=xh_d[:, 2:4, :])
                else:
                    nc.sync.dma_start(
                        out=t[:], in_=xh_d[:, c * XH_CH:(c + 1) * XH_CH, :]
                    )
                xh_chunks.append(t)
            xh_sb = [
                xh_chunks[ko // XH_CH][:, ko % XH_CH, :] for ko in range(KO_RZ)
            ]
            brz_sb = cpool.tile([P, G_RZ], F32, tag="brz")
            nc.sync.dma_start(out=brz_sb[:], in_=brz_d[:])

            bn_sb = cpool.tile([P, G_N], F32, tag="bn")
            bnh_sb = cpool.tile([P, G_N], F32, tag="bnh")

            # Fused r/z projection (16 gate tiles x K=2048), with the
            # n-gate/output-blend work for tile j interleaved after r/z
            # tile 8+j: the serial DVE chain (t -> tanh -> blend) then
            # starts mid-stream and hides under the remaining matmuls
            # instead of pacing a trailing phase of its own.
            rz_blk = rzp.tile([P, G_RZ, BC], F32, tag="rzblk")
            omz_blk = rzp.tile([P, G_N, BC], F32, tag="omzblk")
            zh_blk = rzp.tile([P, G_N, BC], F32, tag="zhblk")
            hf_sb = rzp.tile([P, G_N, BC], F32, tag="hfblk")
            HB = BC // 2  # elementwise half-batch granularity
            for g in range(G_RZ):
                if g == 0:
                    w = w0
                else:
                    w = wrzp.tile([P, KO_RZ, P], F16, tag="wrz")
                    nc.sync.dma_start(out=w[:], in_=wrz_d[g])
                if g == 4 or g == 6:
                    # fp32 h halves, needed from the z tiles (g >= 8) onward
                    half = (g - 4) // 2
                    nc.sync.dma_start(
                        out=hf_sb[:, half * 4:(half + 1) * 4, :],
                        in_=hf_d[:, half * 4:(half + 1) * 4, :],
                    )
                if g == 6:
                    nc.sync.dma_start(out=bn_sb[:], in_=bn_d[:])
                    nc.sync.dma_start(out=bnh_sb[:], in_=bnh_d[:])
                ps = pp_rz.tile([P, BC], F32, tag="psrz")
                for ko in range(KO_RZ):
                    nc.tensor.matmul(
                        ps[:], w[:, ko, :], xh_sb[ko],
                        start=(ko == 0), stop=(ko == KO_RZ - 1),
                    )
                rz = rz_blk[:, g, :]
                nc.scalar.activation(
                    rz, ps[:], AF.Sigmoid, bias=brz_sb[:, g:g + 1]
                )
                if g < G_N:
                    continue
                # ---- n gate + blend for output tile j = g - 8 ----
                j = g - G_N
                nc.vector.tensor_scalar(
                    omz_blk[:, j, :], rz, -1.0, 1.0, op0=ALU.mult, op1=ALU.add
                )
                nc.vector.tensor_mul(
                    out=zh_blk[:, j, :], in0=rz, in1=hf_sb[:, j, :]
                )
                wh = wnp.tile([P, KO_N, P], F16, tag="wnh")
                nc.sync.dma_start(out=wh[:], in_=wnh_d[j])
                wx = wnp.tile([P, KO_N, P], F16, tag="wnx")
                nc.sync.dma_start(out=wx[:], in_=wnx_d[j])
                psx = pp_x.tile([P, BC], F32, tag="psx")
                psh = pp_h.tile([P, BC], F32, tag="psh")
                for ko in range(KO_N):
                    nc.tensor.matmul(
                        psh[:], wh[:, ko, :], xh_sb[KO_N + ko],
                        start=(ko == 0), stop=(ko == KO_N - 1),
                    )
                for ko in range(KO_N):
                    nc.tensor.matmul(
                        psx[:], wx[:, ko, :], xh_sb[ko],
                        start=(ko == 0), stop=(ko == KO_N - 1),
                    )
                o = tp.tile([P, BC], F32, tag="o")
                for hb in range(2):
                    s = slice(hb * HB, (hb + 1) * HB)
                    # t = (psh + b_nh) * r    (overlaps the psx matmuls)
                    t = tp.tile([P, HB], F32, tag=f"t{hb}")
                    nc.vector.scalar_tensor_tensor(
                        t[:], psh[:, s], bnh_sb[:, j:j + 1], rz_blk[:, j, s],
                        op0=ALU.add, op1=ALU.mult,
                    )
                    nc.vector.tensor_add(out=t[:], in0=t[:], in1=psx[:, s])
                    # n = tanh(t + b_n)
                    n_t = tp.tile([P, HB], F32, tag=f"n{hb}")
                    nc.scalar.activation(
                        n_t[:], t[:], AF.Tanh, bias=bn_sb[:, j:j + 1]
                    )
                    # out = n*(1-z) + z*h
                    u = tp.tile([P, HB], F32, tag=f"u{hb}")
                    nc.vector.tensor_mul(
                        out=u[:], in0=n_t[:], in1=omz_blk[:, j, s]
                    )
                    nc.vector.tensor_add(
                        out=o[:, s], in0=u[:], in1=zh_blk[:, j, s]
                    )
                nc.sync.dma_start(out=out_d[:, j, :], in_=o[:])

    nc.compile()
    return nc


def prepare_inputs(x, h, W_ih, b_ih, W_rzh, W_nh, b_nh):
    """Host-side packing: shard batch, transpose/concat/cast weights."""
    f16 = np.float16
    # Fused r/z weight: (IN+H, 2H) -> [g, p, ko, mi] tile-major
    wrz_cat = np.concatenate(
        [W_ih[: 2 * H].T, W_rzh.T], axis=0
    ).astype(f16)
    wrz = np.ascontiguousarray(
        wrz_cat.reshape(KO_RZ, P, G_RZ, P).transpose(2, 1, 0, 3)
    )
    wnx = np.ascontiguousarray(
        W_ih[2 * H:].T.astype(f16).reshape(KO_N, P, G_N, P).transpose(2, 1, 0, 3)
    )
    wnh = np.ascontiguousarray(
        W_nh.T.astype(f16).reshape(KO_N, P, G_N, P).transpose(2, 1, 0, 3)
    )
    brz = np.ascontiguousarray(b_ih[: 2 * H].reshape(G_RZ, P).T).astype(np.float32)
    bn = np.ascontiguousarray(b_ih[2 * H:].reshape(G_N, P).T).astype(np.float32)
    bnh = np.ascontiguousarray(b_nh.reshape(G_N, P).T).astype(np.float32)

    xh_catT = np.concatenate([x.T, h.T], axis=0).astype(f16)  # (2048, B)
    hT = np.ascontiguousarray(h.T.astype(np.float32))          # (1024, B)

    in_maps = []
    for c in range(NCORES):
        cols = slice(c * BC, (c + 1) * BC)
        xh_c = np.ascontiguousarray(
            xh_catT[:, cols].reshape(KO_RZ, P, BC).transpose(1, 0, 2)
        )
        hf_c = np.ascontiguousarray(
            hT[:, cols].reshape(G_N, P, BC).transpose(1, 0, 2)
        )
        in_maps.append(
            {
                "xh": xh_c,
                "hf": hf_c,
                "wrz": wrz,
                "wnx": wnx,
                "wnh": wnh,
                "brz": brz,
                "bn": bn,
                "bnh": bnh,
            }
        )
    return in_maps


def assemble_output(results):
    """results: list of per-core dicts with 'outp' [P, G_N, BC] fp32."""
    parts = []
    for c in range(NCORES):
        oc = results[c]["outp"]                       # [128, 8, 512]
        ocT = oc.transpose(1, 0, 2).reshape(H, BC)    # features x batch
        parts.append(np.ascontiguousarray(ocT.T))     # batch x features
    return np.concatenate(parts, axis=0).astype(np.float32)


def kernel(x, h, W_ih, b_ih, W_rzh, W_nh, b_nh):
    x = np.asarray(x, dtype=np.float32)
    h = np.asarray(h, dtype=np.float32)
    W_ih = np.asarray(W_ih, dtype=np.float32)
    b_ih = np.asarray(b_ih, dtype=np.float32)
    W_rzh = np.asarray(W_rzh, dtype=np.float32)
    W_nh = np.asarray(W_nh, dtype=np.float32)
    b_nh = np.asarray(b_nh, dtype=np.float32)

    in_maps = prepare_inputs(x, h, W_ih, b_ih, W_rzh, W_nh, b_nh)
    nc = build_bass()
    res = run_bass_kernel_spmd(nc, in_maps, core_ids=list(range(NCORES)))
    return assemble_output(res.results)


# revision 10
# speedup vs baseline: 1.0850x; 1.0157x over previous
"""Trainium2 Bass kernel for a fused GRU cell.

Reference computation (B=4096, IN=1024, H=1024, all fp32):
    x_proj = x @ W_ih.T + b_ih            # (B, 3H)
    r_x, z_x, n_x = split(x_proj, 3)
    rz_h = h @ W_rzh.T                    # (B, 2H)
    r = sigmoid(r_x + r_h); z = sigmoid(z_x + z_h)
    n = tanh(n_x + r * (h @ W_nh.T + b_nh))
    out = (1-z)*n + z*h

Strategy:
  - Data-parallel over batch across 8 NeuronCores (512 rows each);
    weights replicated (packed host-side into PE-friendly tiles).
  - Transposed layout on chip: features on partitions, batch on the free
    dim, so per-feature biases are per-partition ACT activation biases.
  - r/z projections fused into ONE K=2048 contraction by concatenating
    [x;h] and [W_ih[:2H].T; W_rzh.T] host-side.
  - Matmuls in fp16 (1 cycle/row on PE, 2 bytes of HBM traffic) with
    fp32 PSUM accumulation; everything else fp32.
"""

import numpy as np

import concourse.bass as bass
import concourse.mybir as mybir
import concourse.tile as tile
from concourse import bacc
from concourse.bass_utils import run_bass_kernel_spmd

B, IN, H = 4096, 1024, 1024
NCORES = 8
BC = B // NCORES          # 512 batch rows per core
P = 128

KO_RZ = (IN + H) // P     # 16 contraction subtiles for the fused r/z matmul
G_RZ = 2 * H // P         # 16 gate tiles (0..7 = r, 8..15 = z)
KO_N = IN // P            # 8
G_N = H // P              # 8

F16 = mybir.dt.float16
F32 = mybir.dt.float32
AF = mybir.ActivationFunctionType
ALU = mybir.AluOpType


def build_bass():
    """Build the per-core Bass program (identical on all cores)."""
    nc = bacc.Bacc("TRN2", target_bir_lowering=False, debug=False)

    xh_d = nc.dram_tensor("xh", [P, KO_RZ, BC], F16, kind="ExternalInput")
    hf_d = nc.dram_tensor("hf", [P, G_N, BC], F32, kind="ExternalInput")
    wrz_d = nc.dram_tensor("wrz", [G_RZ, P, KO_RZ, P], F16, kind="ExternalInput")
    wnx_d = nc.dram_tensor("wnx", [G_N, P, KO_N, P], F16, kind="ExternalInput")
    wnh_d = nc.dram_tensor("wnh", [G_N, P, KO_N, P], F16, kind="ExternalInput")
    brz_d = nc.dram_tensor("brz", [P, G_RZ], F32, kind="ExternalInput")
    bn_d = nc.dram_tensor("bn", [P, G_N], F32, kind="ExternalInput")
    bnh_d = nc.dram_tensor("bnh", [P, G_N], F32, kind="ExternalInput")
    out_d = nc.dram_tensor("outp", [P, G_N, BC], F32, kind="ExternalOutput")

    with tile.TileContext(nc) as tc:
        with (
            tc.tile_pool(name="const", bufs=1) as cpool,
            tc.tile_pool(name="wrzp", bufs=4) as wrzp,
            tc.tile_pool(name="wnp", bufs=3) as wnp,
            tc.tile_pool(name="rzp", bufs=1) as rzp,
            tc.tile_pool(name="tmp", bufs=4) as tp,
            tc.tile_pool(name="ps_rz", bufs=3, space="PSUM") as pp_rz,
            tc.tile_pool(name="ps_x", bufs=2, space="PSUM") as pp_x,
            tc.tile_pool(name="ps_h", bufs=2, space="PSUM") as pp_h,
        ):
            # DMA issue order matters: transfers complete roughly in issue
            # order across the HWDGE queues, and the first matmul waits on
            # the first weight tile — so issue it before the activations.
            # Startup critical path: the first matmul needs only the first
            # ko-chunk of the g=0 weight tile and the first xh chunk, so
            # split those DMAs (deps are view-overlap-based, so matmuls on
            # a ko slice wait only for the chunk that covers it).
            # The startup-critical first weight/activation chunks go through
            # the gpsimd (SWDGE) queue — independent of the HWDGE queues
            # that carry the bulk stream, so the first matmul isn't stuck
            # behind the whole startup backlog.
            w0 = wrzp.tile([P, KO_RZ, P], F16, tag="wrz")
            nc.gpsimd.dma_start(out=w0[:, 0:4, :], in_=wrz_d[0, :, 0:4, :])
            XH_CH = 4
            xh_chunks = []
            for c in range(KO_RZ // XH_CH):
                t = cpool.tile([P, XH_CH, BC], F16, tag=f"xh{c}")
                if c == 0:
                    nc.gpsimd.dma_start(out=t[:, 0:2, :], in_=xh_d[:, 0:2, :])
                    nc.sync.dma_start(out=w0[:, 4:, :], in_=wrz_d[0, :, 4:, :])
                    nc.sync.dma_start(out=t[:, 2:4, :], in_=xh_d[:, 2:4, :])
                else:
                    nc.sync.dma_start(
                        out=t[:], in_=xh_d[:, c * XH_CH:(c + 1) * XH_CH, :]
                    )
                xh_chunks.append(t)
            xh_sb = [
                xh_chunks[ko // XH_CH][:, ko % XH_CH, :] for ko in range(KO_RZ)
            ]
            brz_sb = cpool.tile([P, G_RZ], F32, tag="brz")
            nc.sync.dma_start(out=brz_sb[:], in_=brz_d[:])

            bn_sb = cpool.tile([P, G_N], F32, tag="bn")
            bnh_sb = cpool.tile([P, G_N], F32, tag="bnh")

            # Fused r/z projection (16 gate tiles x K=2048), with the
            # n-gate/output-blend work for tile j interleaved after r/z
            # tile 8+j: the serial DVE chain (t -> tanh -> blend) then
            # starts mid-stream and hides under the remaining matmuls
            # instead of pacing a trailing phase of its own.
            rz_blk = rzp.tile([P, G_RZ, BC], F32, tag="rzblk")
            omz_blk = rzp.tile([P, G_N, BC], F32, tag="omzblk")
            zh_blk = rzp.tile([P, G_N, BC], F32, tag="zhblk")
            hf_sb = rzp.tile([P, G_N, BC], F32, tag="hfblk")
            HB = BC // 2  # elementwise half-batch granularity
            for g in range(G_RZ):
                if g == 0:
                    w = w0
                else:
                    w = wrzp.tile([P, KO_RZ, P], F16, tag="wrz")
                    nc.sync.dma_start(out=w[:], in_=wrz_d[g])
                if g == 4 or g == 6:
                    # fp32 h halves, needed from the z tiles (g >= 8) onward
                    half = (g - 4) // 2
                    nc.sync.dma_start(
                        out=hf_sb[:, half * 4:(half + 1) * 4, :],
                        in_=hf_d[:, half * 4:(half + 1) * 4, :],
                    )
                if g == 6:
                    nc.sync.dma_start(out=bn_sb[:], in_=bn_d[:])
                    nc.sync.dma_start(out=bnh_sb[:], in_=bnh_d[:])
                ps = pp_rz.tile([P, BC], F32, tag="psrz")
                for ko in range(KO_RZ):
                    nc.tensor.matmul(
                        ps[:], w[:, ko, :], xh_sb[ko],
                        start=(ko == 0), stop=(ko == KO_RZ - 1),
                    )
                rz = rz_blk[:, g, :]
                nc.scalar.activation(
                    rz, ps[:], AF.Sigmoid, bias=brz_sb[:, g:g + 1]
                )
                if g < G_N:
                    continue
                # ---- n gate + blend for output tile j = g - 8 ----
                j = g - G_N
                nc.vector.tensor_scalar(
                    omz_blk[:, j, :], rz, -1.0, 1.0, op0=ALU.mult, op1=ALU.add
                )
                nc.vector.tensor_mul(
                    out=zh_blk[:, j, :], in0=rz, in1=hf_sb[:, j, :]
                )
                wh = wnp.tile([P, KO_N, P], F16, tag="wnh")
                nc.sync.dma_start(out=wh[:], in_=wnh_d[j])
                wx = wnp.tile([P, KO_N, P], F16, tag="wnx")
                nc.sync.dma_start(out=wx[:], in_=wnx_d[j])
                psx = pp_x.tile([P, BC], F32, tag="psx")
                psh = pp_h.tile([P, BC], F32, tag="psh")
                for ko in range(KO_N):
                    nc.tensor.matmul(
                        psh[:], wh[:, ko, :], xh_sb[KO_N + ko],
                        start=(ko == 0), stop=(ko == KO_N - 1),
                    )
                for ko in range(KO_N):
                    nc.tensor.matmul(
                        psx[:], wx[:, ko, :], xh_sb[ko],
                        start=(ko == 0), stop=(ko == KO_N - 1),
                    )
                o = tp.tile([P, BC], F32, tag="o")
                for hb in range(2):
                    s = slice(hb * HB, (hb + 1) * HB)
                    # t = (psh + b_nh) * r    (overlaps the psx matmuls)
                    t = tp.tile([P, HB], F32, tag=f"t{hb}")
                    nc.vector.scalar_tensor_tensor(
                        t[:], psh[:, s], bnh_sb[:, j:j + 1], rz_blk[:, j, s],
                        op0=ALU.add, op1=ALU.mult,
                    )
                    nc.vector.tensor_add(out=t[:], in0=t[:], in1=psx[:, s])
                    # n = tanh(t + b_n)
                    n_t = tp.tile([P, HB], F32, tag=f"n{hb}")
                    nc.scalar.activation(
                        n_t[:], t[:], AF.Tanh, bias=bn_sb[:, j:j + 1]
                    )
                    # out = n*(1-z) + z*h
                    u = tp.tile([P, HB], F32, tag=f"u{hb}")
                    nc.vector.tensor_mul(
                        out=u[:], in0=n_t[:], in1=omz_blk[:, j, s]
                    )
                    nc.vector.tensor_add(
                        out=o[:, s], in0=u[:], in1=zh_blk[:, j, s]
                    )
                nc.sync.dma_start(out=out_d[:, j, :], in_=o[:])

    nc.compile()
    return nc


def prepare_inputs(x, h, W_ih, b_ih, W_rzh, W_nh, b_nh):
    """Host-side packing: shard batch, transpose/concat/cast weights."""
    f16 = np.float16
    # Fused r/z weight: (IN+H, 2H) -> [g, p, ko, mi] tile-major
    wrz_cat = np.concatenate(
        [W_ih[: 2 * H].T, W_rzh.T], axis=0
    ).astype(f16)
    wrz = np.ascontiguousarray(
        wrz_cat.reshape(KO_RZ, P, G_RZ, P).transpose(2, 1, 0, 3)
    )
    wnx = np.ascontiguousarray(
        W_ih[2 * H:].T.astype(f16).reshape(KO_N, P, G_N, P).transpose(2, 1, 0, 3)
    )
    wnh = np.ascontiguousarray(
        W_nh.T.astype(f16).reshape(KO_N, P, G_N, P).transpose(2, 1, 0, 3)
    )
    brz = np.ascontiguousarray(b_ih[: 2 * H].reshape(G_RZ, P).T).astype(np.float32)
    bn = np.ascontiguousarray(b_ih[2 * H:].reshape(G_N, P).T).astype(np.float32)
    bnh = np.ascontiguousarray(b_nh.reshape(G_N, P).T).astype(np.float32)

    xh_catT = np.concatenate([x.T, h.T], axis=0).astype(f16)  # (2048, B)
    hT = np.ascontiguousarray(h.T.astype(np.float32))          # (1024, B)

    in_maps = []
    for c in range(NCORES):
        cols = slice(c * BC, (c + 1) * BC)
        xh_c = np.ascontiguousarray(
            xh_catT[:, cols].reshape(KO_RZ, P, BC).transpose(1, 0, 2)
        )
        hf_c = np.ascontiguousarray(
            hT[:, cols].reshape(G_N, P, BC).transpose(1, 0, 2)
        )
        in_maps.append(
            {
                "xh": xh_c,
                "hf": hf_c,
                "wrz": wrz,
                "wnx": wnx,
                "wnh": wnh,
                "brz": brz,
                "bn": bn,
                "bnh": bnh,
            }
        )
    return in_maps


def assemble_output(results):
    """results: list of per-core dicts with 'outp' [P, G_N, BC] fp32."""
    parts = []
    for c in range(NCORES):
        oc = results[c]["outp"]                       # [128, 8, 512]
        ocT = oc.transpose(1, 0, 2).reshape(H, BC)    # features x batch
        parts.append(np.ascontiguousarray(ocT.T))     # batch x features
    return np.concatenate(parts, axis=0).astype(np.float32)


def kernel(x, h, W_ih, b_ih, W_rzh, W_nh, b_nh):
    x = np.asarray(x, dtype=np.float32)
    h = np.asarray(h, dtype=np.float32)
    W_ih = np.asarray(W_ih, dtype=np.float32)
    b_ih = np.asarray(b_ih, dtype=np.float32)
    W_rzh = np.asarray(W_rzh, dtype=np.float32)
    W_nh = np.asarray(W_nh, dtype=np.float32)
    b_nh = np.asarray(b_nh, dtype=np.float32)

    in_maps = prepare_inputs(x, h, W_ih, b_ih, W_rzh, W_nh, b_nh)
    nc = build_bass()
    res = run_bass_kernel_spmd(nc, in_maps, core_ids=list(range(NCORES)))
    return assemble_output(res.results)
